# revision 47
# baseline (speedup 1.0000x reference)
"""Grok1-style attention on 8 trn2 NeuronCores, tensor-parallel over heads.

Sharding (per core c of 8): q heads 4c..4c+3, kv head c; w_qkv column-sharded,
w_o row-sharded; partial o_proj outputs summed on host (the all-reduce).

v11 (baseline 478.7us -> 424.8us measured): software-pipelined single-pass
schedule tuned against measured axon-trn2 behavior (PE warm MM N=512 =
216ns; per-core HBM DMA pool only ~170GB/s, sync ring fastest; ~10us kernel
preamble before the first DMA can issue):
  - causal mask folded into the scores matmul: a [128,128] triangular-const
    matmul pre-adds -2^30 into the PSUM bank (start=True), the scores matmul
    accumulates on top; tanh saturates to -1 -> exp(-30) ~ 1e-13. No DVE
    mask multiplies, no mask tensor.
  - k-tiles processed in pairs sharing a [128,1024] 2-bank PSUM tile: one
    tanh + one exp per pair amortizes the 352-cycle ACT overhead; stale gap
    columns in diagonal pairs saturate through tanh (finite, never read).
  - attnV matmuls run one pair behind the scores so their exp operand is
    ready when the in-order PE queue reaches them.
  - softmax denominator accumulated in bf16 (DVE 2x; +0.5e-3 rel err) and
    1/d via reciprocal_approx_fast off PSUM; finalize chain d-matmul ->
    recip -> partition_broadcast -> mul, with d_ps parked in the idle
    a-bank so the next sweep's matmuls never wait on the recip read.
  - DMA-roofline front: inputs stream on the sync HWDGE ring in exact
    consumption order as 0.5MB tiles (tile-granular dependency tracking);
    qkv0 runs as a 4-feature pass (k,v,q0,q1) + deferred q2,q3 pass with
    window-0 sweeps woven between, filling the DMA-bound start with PE work.
  - qkv filler in 2-feature groups (2 PSUM banks), FIFO with sentinel-forced
    drains; oproj(1) deferred BEHIND qkv(3) so it pops inside ACT-heavy
    window 3 (needs atq bufs=3); h streamed in [128,4,512] eighths whose
    refill DMAs are enqueued as filler items so issue order follows
    WAR-release order. PSUM: scw0(2) scw1(2) a0 a1 fA fB = 8 banks.
"""
import numpy as np
import ml_dtypes
from contextlib import ExitStack

import concourse.bass as bass
import concourse.mybir as mybir
import concourse.tile as tile
from concourse import bacc
from concourse.bass_utils import run_bass_kernel_spmd

T = 2048
D = 4096
HD = 128
HALF = 64
NCORES = 8
HPC = 4                    # q heads per core
QF = HPC * HD              # 512
NF = QF + 2 * HD           # 768 qkv features per core
NCH = D // 128             # 32 contraction chunks
NQ = NCH // 8              # 4 h-quarters per t-tile
TT = 512                   # t-tile width (matmul moving dim)
NTT = T // TT              # 4
NKT = T // 128             # 16 k-tiles
SCALING = HD ** -0.5
CAP = 30.0
BF = mybir.dt.bfloat16
F32 = mybir.dt.float32
FEATG = [[4, 5], [0, 1], [2, 3]]   # k,v first: next window needs them early


def _emit(nc):
    hT_r = nc.dram_tensor("hC", [128, NTT, NCH, TT], BF, kind="ExternalInput").ap()
    wq_r = nc.dram_tensor("wqG", [128, 3, NCH, 256], BF, kind="ExternalInput").ap()
    wo_r = nc.dram_tensor("woC", [128, HPC, D], BF, kind="ExternalInput").ap()
    cc = nc.dram_tensor("cc", [HD, T], BF, kind="ExternalInput").ap()
    ss = nc.dram_tensor("ss", [HD, T], BF, kind="ExternalInput").ap()
    mT_r = nc.dram_tensor("mT", [128, 128], BF, kind="ExternalInput").ap()
    mR_r = nc.dram_tensor("mR", [128, 128], BF, kind="ExternalInput").ap()
    out = nc.dram_tensor("out", [T, D], BF, kind="ExternalOutput").ap()

    with tile.TileContext(nc) as tc:
        with ExitStack() as ctx:
            wqp = ctx.enter_context(tc.tile_pool(name="wqp", bufs=1))
            hqp = ctx.enter_context(tc.tile_pool(name="hqp", bufs=12))
            cstp = ctx.enter_context(tc.tile_pool(name="cstp", bufs=1))
            seqp = ctx.enter_context(tc.tile_pool(name="seqp", bufs=1))
            qtp = ctx.enter_context(tc.tile_pool(name="qtp", bufs=2))
            atp = ctx.enter_context(tc.tile_pool(name="atp", bufs=3))
            vtp = ctx.enter_context(tc.tile_pool(name="vtp", bufs=2))
            rtp = ctx.enter_context(tc.tile_pool(name="rtp", bufs=2))
            stp = ctx.enter_context(tc.tile_pool(name="stp", bufs=2))
            etp = ctx.enter_context(tc.tile_pool(name="etp", bufs=3))
            accp = ctx.enter_context(tc.tile_pool(name="accp", bufs=2))
            nrmp = ctx.enter_context(tc.tile_pool(name="nrmp", bufs=2))
            bcp = ctx.enter_context(tc.tile_pool(name="bcp", bufs=2))
            obp = ctx.enter_context(tc.tile_pool(name="obp", bufs=4))
            psp = ctx.enter_context(tc.tile_pool(name="psp", bufs=1, space="PSUM"))

            # ---- prelude ----
            # All bulk inputs stream on the sync (qSP HWDGE) ring in exact
            # need-order; measured aggregate DMA is only ~170GB/s here, so
            # the front of the kernel is DMA-roofline bound and ordering is
            # everything. The scalar ring measured ~3x slower; it only gets
            # wo (first needed ~150us in).
            hq = {}  # (tt, i) -> [128, 4, TT] eighth tile (0.5MB)

            def h_eighth_dma(tt, i):
                t = hqp.tile([128, 4, TT], BF, tag="hq", name=f"h{tt}e{i}")
                hq[(tt, i)] = t
                nc.sync.dma_start(t[:], hT_r[:, tt, 4 * i:4 * (i + 1), :])

            # wq group-major: 12 tiles of [128, 8, 256] (0.5MB) per (g, j)
            wqg = {}

            def wq_dma(g, j):
                wqg[(g, j)] = wqp.tile([128, 8, 256], BF, tag=f"wq{g}{j}",
                                       name=f"wq{g}{j}")
                nc.sync.dma_start(wqg[(g, j)][:],
                                  wq_r[:, g, 8 * j:8 * (j + 1), :])

            # per 8-chunk block: both weight groups' tiles land just before
            # the h eighths they pair with, so qkv0 passA (4 features) is
            # DMA-pipelined chunk-by-chunk instead of waiting for all of g1
            for j in range(4):
                wq_dma(0, j)
                wq_dma(1, j)
                h_eighth_dma(0, 2 * j)
                h_eighth_dma(0, 2 * j + 1)
            # consts on the (slow) scalar ring — small and needed by ~50us;
            # wq group 2 follows the main sync stream (passB is delayed past
            # sweep(1,0) so its tail tiles have time to land)
            cc_sb = cstp.tile([HD, T], BF, tag="cc")
            ss_sb = cstp.tile([HD, T], BF, tag="ss")
            nc.scalar.dma_start(cc_sb[:], cc[:, :])
            nc.scalar.dma_start(ss_sb[:], ss[:, :])
            mT_sb = cstp.tile([128, 128], BF, tag="mT")
            mR_sb = cstp.tile([128, 128], BF, tag="mR")
            nc.scalar.dma_start(mT_sb[:], mT_r[:, :])
            nc.scalar.dma_start(mR_sb[:], mR_r[:, :])
            for j in range(4):
                wq_dma(2, j)

            F2G = {4: 0, 5: 0, 0: 1, 1: 1, 2: 2, 3: 2}

            def wq_ap(c, f):
                g = F2G[f]
                col = 128 * FEATG[g].index(f)
                return wqg[(g, c // 8)][:, c % 8, col:col + 128]

            ones_k = cstp.tile([128, 1], BF, tag="ones_k")
            nc.gpsimd.memset(ones_k[:], 1.0)
            wo_sb = cstp.tile([128, HPC, D], BF, tag="wo")

            # persistent per-sequence tiles
            kTt = [seqp.tile([HD, TT], BF, tag=f"k_{tt}", name=f"kT{tt}")
                   for tt in range(NTT)]
            vbt = [seqp.tile([128, HD], BF, tag=f"vb_{kt}", name=f"vb{kt}")
                   for kt in range(NKT)]
            qTt = {}   # (h, qt) -> tile
            atq = {}   # (h, qt) -> tile

            # ---- filler machinery (FIFO + sentinels) ----
            filler = []
            state = {"popped": 0, "enq": 0}
            marks = {}

            def enq(fn):
                filler.append(fn)
                state["enq"] += 1

            def mark(name):
                marks[name] = state["enq"]

            def drain(n):
                for _ in range(min(n, len(filler))):
                    filler.pop(0)()
                    state["popped"] += 1

            def drain_until(name):
                tgt = marks[name]
                while state["popped"] < tgt:
                    filler.pop(0)()
                    state["popped"] += 1

            def drain_all():
                while filler:
                    filler.pop(0)()
                    state["popped"] += 1

            # ---- rope ----
            def rope_copy(tt, f, ps_ap, st_):
                if f != 5:
                    qk_sb = rtp.tile([128, TT], BF, tag=f"qk{f % 2}",
                                     name=f"qk{f}_{tt}")
                    st_[("qk", f)] = qk_sb
                    nc.scalar.copy(qk_sb[:], ps_ap)
                else:
                    vT = vtp.tile([128, TT], BF, tag="vT", name=f"vT{tt}")
                    st_[("qk", f)] = vT
                    nc.scalar.copy(vT[:], ps_ap)

            def rope_rest(tt, f, st_):
                t0 = tt * TT
                if f != 5:
                    qk_sb = st_[("qk", f)]
                    dst = qTt[(f, tt)] if f < HPC else kTt[tt]
                    rot = rtp.tile([128, TT], BF, tag="rot")
                    # SBUF->SBUF shifts ride the (empty) gpsimd DMA ring: on
                    # sync/scalar they queue behind megabytes of HBM stream
                    # and gate the rope chain (6.3us PE gap observed)
                    nc.gpsimd.dma_start(rot[0:HALF, :], qk_sb[HALF:128, :])
                    nc.gpsimd.dma_start(rot[HALF:128, :], qk_sb[0:HALF, :])
                    m1 = rtp.tile([128, TT], BF, tag="m1")
                    nc.vector.tensor_mul(m1[:], qk_sb[:], cc_sb[:, t0:t0 + TT])
                    m2 = rtp.tile([128, TT], BF, tag="m2")
                    nc.vector.tensor_mul(m2[:], rot[:], ss_sb[:, t0:t0 + TT])
                    nc.vector.tensor_add(dst[:], m1[:], m2[:])
                else:
                    vT = st_[("qk", f)]
                    for i in range(4):
                        # xbar transpose needs a HWDGE ring; scalar is far
                        # less queued than sync when these fire
                        nc.scalar.dma_start_transpose(
                            vbt[4 * tt + i][:], vT[:, i * 128:(i + 1) * 128])

            # ---- qkv filler items: 3 groups of 2 features, 2 psum banks ----
            def qkv_items(tt):
                for h in range(HPC):
                    qTt[(h, tt)] = qtp.tile([HD, TT], BF, tag=f"q{h}",
                                            name=f"qT{h}_{tt}")
                for g in range(3):
                    st_ = {}

                    def mk_mm(g, c, st_=st_):
                        def mm():
                            if c == 0:
                                st_["ps2"] = [
                                    psp.tile([128, TT], F32, tag=t,
                                             name=f"qkv{tt}g{g}_{t}")
                                    for t in ("fA", "fB")]
                            src = hq[(tt, c // 4)][:, c % 4, :]
                            for j in range(2):
                                f = FEATG[g][j]
                                nc.tensor.matmul(
                                    st_["ps2"][j][:], wq_ap(c, f), src,
                                    start=(c == 0), stop=(c == NCH - 1),
                                )
                        return mm

                    for c in range(NCH):
                        enq(mk_mm(g, c))
                        # h(tt+1) eighth DMAs ride the last group's pops so
                        # issue order matches WAR-release order on the ring
                        if g == 2 and tt < NTT - 1 and c % 4 == 3:
                            enq(lambda tt=tt, i=c // 4: h_eighth_dma(tt + 1, i))
                    for j in range(2):
                        enq(lambda g=g, j=j, st_=st_: rope_copy(
                            tt, FEATG[g][j], st_["ps2"][j][:], st_))
                    for j in range(2):
                        enq(lambda g=g, j=j, st_=st_: rope_rest(
                            tt, FEATG[g][j], st_))
                    mark(f"qkv{tt}_g{g}")

            # ---- qkv(0): inline passes, chunk-outer over given features ----
            def qkv0_pass(feats, dst_fn):
                dsts = dst_fn()
                for c in range(NCH):
                    src = hq[(0, c // 4)][:, c % 4, :]
                    for idx in range(len(feats)):
                        nc.tensor.matmul(
                            dsts[idx], wq_ap(c, feats[idx]), src,
                            start=(c == 0), stop=(c == NCH - 1),
                        )
                st_ = {}
                for idx, f in enumerate(feats):
                    rope_copy(0, f, dsts[idx], st_)
                    rope_rest(0, f, st_)

            # ---- o_proj items ----
            def oproj_items(qt, ob_eng, ring="sync"):
                for t16 in range(4 * qt, 4 * qt + 4):
                    for nb in range(8):
                        n0 = nb * TT
                        k = t16 * 8 + nb

                        def op(t16=t16, n0=n0, k=k):
                            o_ps = psp.tile([128, TT], F32,
                                            tag=("fA", "fB")[k % 2], name="o_ps")
                            for fc in range(HPC):
                                lhsT = atq[(fc, t16 // 4)][
                                    :, (t16 % 4) * 128:(t16 % 4 + 1) * 128]
                                nc.tensor.matmul(
                                    o_ps[:], lhsT, wo_sb[:, fc, n0:n0 + TT],
                                    start=(fc == 0), stop=(fc == HPC - 1),
                                )
                            ob = obp.tile([128, TT], BF, tag="ob")
                            if ob_eng == "mix" and k % 2 == 0:
                                nc.vector.tensor_copy(ob[:], o_ps[:])
                            elif ob_eng == "mix":
                                nc.scalar.copy(ob[:], o_ps[:])
                            else:
                                nc.vector.tensor_copy(ob[:], o_ps[:])
                            eng = (nc.sync if ring == "sync" or k % 2 == 0
                                   else nc.scalar)
                            eng.dma_start(
                                out[t16 * 128:(t16 + 1) * 128, n0:n0 + TT], ob[:])
                        enq(op)
                mark(f"oproj{qt}")

            # ---- attention sweep ----
            sweep_no = [0]

            def sweep(h, qt):
                sid = sweep_no[0]
                sweep_no[0] += 1
                qT = qTt[(h, qt)]
                atq[(h, qt)] = atp.tile([HD, TT], BF, tag=f"at{h}",
                                        name=f"at{h}_{qt}")
                acc = accp.tile([128, TT], BF, tag=f"acc{sid % 2}",
                                name=f"acc{h}_{qt}")
                a_ps = psp.tile([HD, TT], F32, tag=f"a{sid % 2}",
                                name=f"a_ps{h}_{qt}")
                nkt = 4 * qt + 4
                # k-tiles processed in pairs sharing a [128,1024] 2-bank PSUM
                # tile so tanh/exp amortize the ~293ns/op ACT overhead; attnV
                # runs one pair behind so its et operand is always ready.
                # Diag pairs: the gap columns hold stale garbage that tanh
                # saturates to +-1 (finite) and nothing downstream reads.
                prev = None
                for p in range(nkt // 2):
                    s_w = psp.tile([128, 2 * TT], F32, tag=f"scw{p % 2}",
                                   name="s_w")
                    halves = []
                    for half in range(2):
                        kt = 2 * p + half
                        m = kt - 4 * qt
                        j0 = 128 * m if m > 0 else 0
                        off = half * TT
                        ks = kTt[kt // 4][:, (kt % 4) * 128:(kt % 4 + 1) * 128]
                        if m >= 0:
                            nc.tensor.matmul(
                                s_w[:, off + j0:off + j0 + 128], mT_sb[:],
                                mR_sb[:], start=True, stop=False)
                            nc.tensor.matmul(
                                s_w[:, off + j0:off + TT], ks, qT[:, j0:TT],
                                start=False, stop=True)
                        else:
                            nc.tensor.matmul(s_w[:, off:off + TT], ks, qT[:],
                                             start=True, stop=True)
                        halves.append((j0, kt, off))
                    ja = halves[0][0]
                    st = stp.tile([128, 2 * TT], F32, tag="st")
                    nc.scalar.activation(
                        st[:, ja:2 * TT], s_w[:, ja:2 * TT],
                        mybir.ActivationFunctionType.Tanh,
                        scale=SCALING / CAP,
                    )
                    et_w = etp.tile([128, 2 * TT], BF, tag="et")
                    nc.scalar.activation(
                        et_w[:, ja:2 * TT], st[:, ja:2 * TT],
                        mybir.ActivationFunctionType.Exp,
                        scale=CAP,
                    )
                    for (j0, kt, off) in halves:
                        if kt == 0:
                            nc.vector.tensor_copy(acc[:], et_w[:, 0:TT])
                        else:
                            nc.vector.tensor_add(
                                acc[:, j0:TT], acc[:, j0:TT],
                                et_w[:, off + j0:off + TT])
                    if prev is not None:
                        pet, ph = prev
                        for (j0, kt, off) in ph:
                            nc.tensor.matmul(
                                a_ps[:, j0:TT], vbt[kt][:],
                                pet[:, off + j0:off + TT],
                                start=(kt == 0), stop=False,
                            )
                    prev = (et_w, halves)
                    drain(3 if halves[0][1] < 4 * qt else 2)
                pet, ph = prev
                drain(1)
                for idx, (j0, kt, off) in enumerate(ph):
                    nc.tensor.matmul(
                        a_ps[:, j0:TT], vbt[kt][:], pet[:, off + j0:off + TT],
                        start=(kt == 0), stop=(idx == len(ph) - 1),
                    )
                drain(2)
                # finalize: d = ones^T acc; 1/d; broadcast; normalize into atq
                # d_ps rides the OTHER a-slot: free since the previous
                # finalize's mul, and its next writer (next sweep's pair-1
                # attnV, ~3us in) comfortably outwaits the recip read.
                d_ps = psp.tile([1, TT], F32, tag=f"a{(sid + 1) % 2}",
                                name=f"d{h}_{qt}")
                nc.tensor.matmul(d_ps[:], ones_k[:], acc[:],
                                 start=True, stop=True)
                rc = nrmp.tile([1, TT], F32, tag="rc", name=f"rc{h}_{qt}")
                nc.vector.reciprocal_approx_fast(rc[:], d_ps[:])
                bcrc = bcp.tile([128, TT], F32, tag="bcrc", name=f"bc{h}_{qt}")
                nc.gpsimd.partition_broadcast(bcrc[:], rc[:])
                nc.vector.tensor_mul(atq[(h, qt)][:], a_ps[:], bcrc[:])

            # ================= main schedule =================
            # front: qkv0 in two passes with window-0 sweeps woven between
            # (attention PE work fills the DMA-bound front)
            for h in range(HPC):
                qTt[(h, 0)] = qtp.tile([HD, TT], BF, tag=f"q{h}",
                                       name=f"qT{h}_0")
            def _dstsA():
                pfA = psp.tile([128, TT], F32, tag="fA", name="qkv0_fA")
                pfB = psp.tile([128, TT], F32, tag="fB", name="qkv0_fB")
                pw = psp.tile([128, 2 * TT], F32, tag="scw0", name="qkv0_w")
                return [pfA[:], pfB[:], pw[:, 0:TT], pw[:, TT:2 * TT]]

            def _dstsB():
                pfA = psp.tile([128, TT], F32, tag="fA", name="qkv0b_fA")
                pfB = psp.tile([128, TT], F32, tag="fB", name="qkv0b_fB")
                return [pfA[:], pfB[:]]

            qkv0_pass([4, 5, 0, 1], _dstsA)
            sweep(0, 0)
            sweep(1, 0)
            qkv0_pass([2, 3], _dstsB)
            for i in range(8):
                h_eighth_dma(1, i)
            # wo on the (otherwise idle) scalar ring; first needed ~150us in
            for j in range(2):
                nc.scalar.dma_start(
                    wo_sb[:, 2 * j:2 * j + 2, :], wo_r[:, 2 * j:2 * j + 2, :])
            qkv_items(1)
            sweep(2, 0)
            sweep(3, 0)
            oproj_items(0, "mix")

            # window 1
            qkv_items(2)
            drain_until("qkv1_g1")
            sweep(0, 1)
            sweep(1, 1)
            drain_until("qkv1_g2")
            sweep(2, 1)
            sweep(3, 1)

            # window 2. oproj(1) is deferred BEHIND qkv(3) in the FIFO so it
            # pops inside ACT-heavy window 3, which otherwise starves the PE
            # (needs atp bufs=3: atq(*,1) must survive into window 3).
            qkv_items(3)
            oproj_items(1, "dve", ring="split")
            drain_until("qkv2_g1")
            sweep(0, 2)
            sweep(1, 2)
            drain_until("qkv2_g2")
            sweep(2, 2)
            sweep(3, 2)
            oproj_items(2, "dve", ring="split")

            # window 3
            drain_until("qkv3_g1")
            sweep(0, 3)
            sweep(1, 3)
            drain_until("qkv3_g2")
            sweep(2, 3)
            sweep(3, 3)

            drain_all()
            oproj_items(3, "mix", ring="split")
            drain_all()
    return nc


_CACHE = {}


def _get_nc():
    if "nc" not in _CACHE:
        nc = bacc.Bacc("TRN2", target_bir_lowering=False, debug=False)
        _emit(nc)
        nc.compile()
        _CACHE["nc"] = nc
    return _CACHE["nc"]


def _in_maps(positions, hidden_states, w_qkv, w_o):
    bf16 = ml_dtypes.bfloat16
    hidden_states = np.asarray(hidden_states, dtype=np.float32)
    w_qkv = np.asarray(w_qkv, dtype=np.float32)
    w_o = np.asarray(w_o, dtype=np.float32)
    pos = np.asarray(positions).astype(np.float64)

    # hC[p, tt, c, t] = hidden.T[c*128+p, tt*512+t]
    hT = np.ascontiguousarray(hidden_states.T).astype(bf16)      # [D, T]
    hC = np.ascontiguousarray(
        hT.reshape(NCH, 128, NTT, TT).transpose(1, 2, 0, 3))     # [128,4,32,512]
    inv_freq = 1.0 / (10000.0 ** (np.arange(HALF, dtype=np.float64) * 2.0 / HD))
    ang = np.outer(inv_freq, pos)                      # [64, T]
    cos = np.cos(ang).astype(np.float32)
    sin = np.sin(ang).astype(np.float32)
    ccm = np.ascontiguousarray(np.concatenate([cos, cos], axis=0)).astype(bf16)
    ssm = np.ascontiguousarray(np.concatenate([-sin, sin], axis=0)).astype(bf16)
    # mask consts: out[i,t] = sum_k mT[k,i]*mR[k,t] = -2^30 iff t < i
    mTm = np.ascontiguousarray(np.triu(np.ones((128, 128)))).astype(bf16)
    mRm = np.zeros((128, 128), dtype=np.float32)
    mRm[np.arange(1, 128), np.arange(0, 127)] = -float(2 ** 30)
    mRm = np.ascontiguousarray(mRm).astype(bf16)

    in_maps = []
    for c in range(NCORES):
        rows = np.concatenate([
            w_qkv[QF * c:QF * (c + 1)],
            w_qkv[D + HD * c:D + HD * (c + 1)],
            w_qkv[D + HD * NCORES + HD * c:D + HD * NCORES + HD * (c + 1)],
        ], axis=0)                                      # [768, 4096]
        wq_c = rows.T.astype(bf16)                      # [4096, 768]
        # group-major feature order [k,v | q0,q1 | q2,q3] so each group's
        # weight columns are one contiguous DMA stream
        perm = np.r_[512:768, 0:256, 256:512]
        wqC = np.ascontiguousarray(
            wq_c[:, perm].reshape(NCH, 128, 3, 256)
            .transpose(1, 2, 0, 3))                     # [128,3,32,256]
        wo_c = w_o[:, QF * c:QF * (c + 1)].T.astype(bf16)        # [512, 4096]
        woC = np.ascontiguousarray(
            wo_c.reshape(HPC, 128, D).transpose(1, 0, 2))        # [128,4,4096]
        in_maps.append({"hC": hC, "wqG": wqC, "woC": woC,
                        "cc": ccm, "ss": ssm, "mT": mTm, "mR": mRm})
    return in_maps


def run(positions, hidden_states, w_qkv, w_o, trace=False):
    nc = _get_nc()
    in_maps = _in_maps(positions, hidden_states, w_qkv, w_o)
    res = run_bass_kernel_spmd(nc, in_maps, list(range(NCORES)), trace=trace)
    parts = np.stack([np.asarray(res.results[i]["out"], dtype=np.float32)
                      for i in range(NCORES)], axis=0)
    full = parts.sum(axis=0, dtype=np.float64).astype(np.float32)
    return full, res


def kernel(positions, hidden_states, w_qkv, w_o):
    full, _ = run(positions, hidden_states, w_qkv, w_o, trace=False)
    return full


# revision 51
# speedup vs baseline: 1.0348x; 1.0348x over previous
"""Grok1-style attention on 8 trn2 NeuronCores, tensor-parallel over heads.

Sharding (per core c of 8): q heads 4c..4c+3, kv head c; w_qkv column-sharded,
w_o row-sharded; partial o_proj outputs summed on host (the all-reduce).

v11 (baseline 478.7us -> 424.8us measured): software-pipelined single-pass
schedule tuned against measured axon-trn2 behavior (PE warm MM N=512 =
216ns; per-core HBM DMA pool only ~170GB/s, sync ring fastest; ~10us kernel
preamble before the first DMA can issue):
  - causal mask folded into the scores matmul: a [128,128] triangular-const
    matmul pre-adds -2^30 into the PSUM bank (start=True), the scores matmul
    accumulates on top; tanh saturates to -1 -> exp(-30) ~ 1e-13. No DVE
    mask multiplies, no mask tensor.
  - k-tiles processed in pairs sharing a [128,1024] 2-bank PSUM tile: one
    tanh + one exp per pair amortizes the 352-cycle ACT overhead; stale gap
    columns in diagonal pairs saturate through tanh (finite, never read).
  - attnV matmuls run one pair behind the scores so their exp operand is
    ready when the in-order PE queue reaches them.
  - softmax denominator accumulated in bf16 (DVE 2x; +0.5e-3 rel err) and
    1/d via reciprocal_approx_fast off PSUM; finalize chain d-matmul ->
    recip -> partition_broadcast -> mul, with d_ps parked in the idle
    a-bank so the next sweep's matmuls never wait on the recip read.
  - DMA-roofline front: inputs stream on the sync HWDGE ring in exact
    consumption order as 0.5MB tiles (tile-granular dependency tracking);
    qkv0 runs as a 4-feature pass (k,v,q0,q1) + deferred q2,q3 pass with
    window-0 sweeps woven between, filling the DMA-bound start with PE work.
  - qkv filler in 2-feature groups (2 PSUM banks), FIFO with sentinel-forced
    drains; oproj(1) deferred BEHIND qkv(3) so it pops inside ACT-heavy
    window 3 (needs atq bufs=3); h streamed in [128,4,512] eighths whose
    refill DMAs are enqueued as filler items so issue order follows
    WAR-release order. PSUM: scw0(2) scw1(2) a0 a1 fA fB = 8 banks.
"""
import numpy as np
import ml_dtypes
from contextlib import ExitStack

import concourse.bass as bass
import concourse.mybir as mybir
import concourse.tile as tile
from concourse import bacc
from concourse.bass_utils import run_bass_kernel_spmd

T = 2048
D = 4096
HD = 128
HALF = 64
NCORES = 8
HPC = 4                    # q heads per core
QF = HPC * HD              # 512
NF = QF + 2 * HD           # 768 qkv features per core
NCH = D // 128             # 32 contraction chunks
NQ = NCH // 8              # 4 h-quarters per t-tile
TT = 512                   # t-tile width (matmul moving dim)
NTT = T // TT              # 4
NKT = T // 128             # 16 k-tiles
SCALING = HD ** -0.5
CAP = 30.0
BF = mybir.dt.bfloat16
F32 = mybir.dt.float32
FEATG = [[4, 5], [0, 1], [2, 3]]   # k,v first: next window needs them early


def _emit(nc):
    hT_r = nc.dram_tensor("hC", [128, NTT, NCH, TT], BF, kind="ExternalInput").ap()
    wq_r = nc.dram_tensor("wqG", [128, 3, NCH, 256], BF, kind="ExternalInput").ap()
    wo_r = nc.dram_tensor("woC", [128, HPC, D], BF, kind="ExternalInput").ap()
    cc = nc.dram_tensor("cc", [HD, T], BF, kind="ExternalInput").ap()
    ss = nc.dram_tensor("ss", [HD, T], BF, kind="ExternalInput").ap()
    mT_r = nc.dram_tensor("mT", [128, 128], BF, kind="ExternalInput").ap()
    mR_r = nc.dram_tensor("mR", [128, 128], BF, kind="ExternalInput").ap()
    out = nc.dram_tensor("out", [T, D], BF, kind="ExternalOutput").ap()

    with tile.TileContext(nc) as tc:
        with ExitStack() as ctx:
            wqp = ctx.enter_context(tc.tile_pool(name="wqp", bufs=1))
            hqp = ctx.enter_context(tc.tile_pool(name="hqp", bufs=12))
            cstp = ctx.enter_context(tc.tile_pool(name="cstp", bufs=1))
            seqp = ctx.enter_context(tc.tile_pool(name="seqp", bufs=1))
            qtp = ctx.enter_context(tc.tile_pool(name="qtp", bufs=2))
            atp = ctx.enter_context(tc.tile_pool(name="atp", bufs=3))
            vtp = ctx.enter_context(tc.tile_pool(name="vtp", bufs=2))
            rtp = ctx.enter_context(tc.tile_pool(name="rtp", bufs=2))
            stp = ctx.enter_context(tc.tile_pool(name="stp", bufs=2))
            etp = ctx.enter_context(tc.tile_pool(name="etp", bufs=3))
            accp = ctx.enter_context(tc.tile_pool(name="accp", bufs=2))
            nrmp = ctx.enter_context(tc.tile_pool(name="nrmp", bufs=2))
            bcp = ctx.enter_context(tc.tile_pool(name="bcp", bufs=2))
            obp = ctx.enter_context(tc.tile_pool(name="obp", bufs=4))
            psp = ctx.enter_context(tc.tile_pool(name="psp", bufs=1, space="PSUM"))

            # ---- prelude ----
            # All bulk inputs stream on the sync (qSP HWDGE) ring in exact
            # need-order; measured aggregate DMA is only ~170GB/s here, so
            # the front of the kernel is DMA-roofline bound and ordering is
            # everything. The scalar ring measured ~3x slower; it only gets
            # wo (first needed ~150us in).
            hq = {}  # (tt, i) -> [128, 4, TT] eighth tile (0.5MB)

            def h_eighth_dma(tt, i, eng=None):
                t = hqp.tile([128, 4, TT], BF, tag="hq", name=f"h{tt}e{i}")
                hq[(tt, i)] = t
                (eng or nc.sync).dma_start(
                    t[:], hT_r[:, tt, 4 * i:4 * (i + 1), :])

            # wq group-major: 12 tiles of [128, 8, 256] (0.5MB) per (g, j)
            wqg = {}

            def wq_dma(g, j, eng=None):
                wqg[(g, j)] = wqp.tile([128, 8, 256], BF, tag=f"wq{g}{j}",
                                       name=f"wq{g}{j}")
                (eng or nc.sync).dma_start(wqg[(g, j)][:],
                                           wq_r[:, g, 8 * j:8 * (j + 1), :])

            # per 8-chunk block: both weight groups' tiles land just before
            # the h eighths they pair with, so qkv0 passA (4 features) is
            # DMA-pipelined chunk-by-chunk instead of waiting for all of g1.
            # The last block's tiles (needed ~50-60us in) ride the otherwise
            # idle gpsimd SWDGE ring (~40GB/s), trimming the sync stream that
            # paces passA by ~1.5MB.
            h_eighth_dma(0, 6, eng=nc.gpsimd)
            wq_dma(0, 3, eng=nc.gpsimd)
            h_eighth_dma(0, 7, eng=nc.gpsimd)
            for j in range(3):
                wq_dma(0, j)
                wq_dma(1, j)
                h_eighth_dma(0, 2 * j)
                h_eighth_dma(0, 2 * j + 1)
            wq_dma(1, 3)
            # consts on the (slow) scalar ring — small and needed by ~50us;
            # wq group 2 follows the main sync stream (passB is delayed past
            # sweep(1,0) so its tail tiles have time to land)
            cc_sb = cstp.tile([HD, T], BF, tag="cc")
            ss_sb = cstp.tile([HD, T], BF, tag="ss")
            nc.scalar.dma_start(cc_sb[:], cc[:, :])
            nc.scalar.dma_start(ss_sb[:], ss[:, :])
            mT_sb = cstp.tile([128, 128], BF, tag="mT")
            mR_sb = cstp.tile([128, 128], BF, tag="mR")
            nc.scalar.dma_start(mT_sb[:], mT_r[:, :])
            nc.scalar.dma_start(mR_sb[:], mR_r[:, :])
            for j in range(4):
                wq_dma(2, j)

            F2G = {4: 0, 5: 0, 0: 1, 1: 1, 2: 2, 3: 2}

            def wq_ap(c, f):
                g = F2G[f]
                col = 128 * FEATG[g].index(f)
                return wqg[(g, c // 8)][:, c % 8, col:col + 128]

            ones_k = cstp.tile([128, 1], BF, tag="ones_k")
            nc.gpsimd.memset(ones_k[:], 1.0)
            wo_sb = cstp.tile([128, HPC, D], BF, tag="wo")

            # persistent per-sequence tiles
            kTt = [seqp.tile([HD, TT], BF, tag=f"k_{tt}", name=f"kT{tt}")
                   for tt in range(NTT)]
            vbt = [seqp.tile([128, HD], BF, tag=f"vb_{kt}", name=f"vb{kt}")
                   for kt in range(NKT)]
            qTt = {}   # (h, qt) -> tile
            atq = {}   # (h, qt) -> tile

            # ---- filler machinery (FIFO + sentinels) ----
            filler = []
            state = {"popped": 0, "enq": 0}
            marks = {}

            def enq(fn):
                filler.append(fn)
                state["enq"] += 1

            def mark(name):
                marks[name] = state["enq"]

            def drain(n):
                for _ in range(min(n, len(filler))):
                    filler.pop(0)()
                    state["popped"] += 1

            def drain_until(name):
                tgt = marks[name]
                while state["popped"] < tgt:
                    filler.pop(0)()
                    state["popped"] += 1

            def drain_all():
                while filler:
                    filler.pop(0)()
                    state["popped"] += 1

            # ---- rope ----
            def rope_copy(tt, f, ps_ap, st_):
                if f != 5:
                    qk_sb = rtp.tile([128, TT], BF, tag=f"qk{f % 2}",
                                     name=f"qk{f}_{tt}")
                    st_[("qk", f)] = qk_sb
                    nc.scalar.copy(qk_sb[:], ps_ap)
                else:
                    vT = vtp.tile([128, TT], BF, tag="vT", name=f"vT{tt}")
                    st_[("qk", f)] = vT
                    nc.scalar.copy(vT[:], ps_ap)

            def rope_rest(tt, f, st_):
                t0 = tt * TT
                if f != 5:
                    qk_sb = st_[("qk", f)]
                    dst = qTt[(f, tt)] if f < HPC else kTt[tt]
                    rot = rtp.tile([128, TT], BF, tag="rot")
                    nc.sync.dma_start(rot[0:HALF, :], qk_sb[HALF:128, :])
                    nc.sync.dma_start(rot[HALF:128, :], qk_sb[0:HALF, :])
                    m1 = rtp.tile([128, TT], BF, tag="m1")
                    nc.vector.tensor_mul(m1[:], qk_sb[:], cc_sb[:, t0:t0 + TT])
                    m2 = rtp.tile([128, TT], BF, tag="m2")
                    nc.vector.tensor_mul(m2[:], rot[:], ss_sb[:, t0:t0 + TT])
                    nc.vector.tensor_add(dst[:], m1[:], m2[:])
                else:
                    vT = st_[("qk", f)]
                    for i in range(4):
                        nc.sync.dma_start_transpose(
                            vbt[4 * tt + i][:], vT[:, i * 128:(i + 1) * 128])

            # ---- qkv filler items: 3 groups of 2 features, 2 psum banks ----
            def qkv_items(tt):
                for h in range(HPC):
                    qTt[(h, tt)] = qtp.tile([HD, TT], BF, tag=f"q{h}",
                                            name=f"qT{h}_{tt}")
                for g in range(3):
                    st_ = {}

                    def mk_mm(g, c, st_=st_):
                        def mm():
                            if c == 0:
                                st_["ps2"] = [
                                    psp.tile([128, TT], F32, tag=t,
                                             name=f"qkv{tt}g{g}_{t}")
                                    for t in ("fA", "fB")]
                            src = hq[(tt, c // 4)][:, c % 4, :]
                            for j in range(2):
                                f = FEATG[g][j]
                                nc.tensor.matmul(
                                    st_["ps2"][j][:], wq_ap(c, f), src,
                                    start=(c == 0), stop=(c == NCH - 1),
                                )
                        return mm

                    for c in range(NCH):
                        enq(mk_mm(g, c))
                        # h(tt+1) eighth DMAs ride the last group's pops so
                        # issue order matches WAR-release order on the ring
                        if g == 2 and tt < NTT - 1 and c % 4 == 3:
                            enq(lambda tt=tt, i=c // 4: h_eighth_dma(tt + 1, i))
                    for j in range(2):
                        enq(lambda g=g, j=j, st_=st_: rope_copy(
                            tt, FEATG[g][j], st_["ps2"][j][:], st_))
                    for j in range(2):
                        enq(lambda g=g, j=j, st_=st_: rope_rest(
                            tt, FEATG[g][j], st_))
                    mark(f"qkv{tt}_g{g}")

            # ---- qkv(0): inline passes, chunk-outer over given features ----
            def qkv0_pass(feats, dst_fn):
                dsts = dst_fn()
                for c in range(NCH):
                    src = hq[(0, c // 4)][:, c % 4, :]
                    for idx in range(len(feats)):
                        nc.tensor.matmul(
                            dsts[idx], wq_ap(c, feats[idx]), src,
                            start=(c == 0), stop=(c == NCH - 1),
                        )
                st_ = {}
                for idx, f in enumerate(feats):
                    rope_copy(0, f, dsts[idx], st_)
                    rope_rest(0, f, st_)

            # ---- o_proj items ----
            def oproj_items(qt, ob_eng, ring="sync"):
                for t16 in range(4 * qt, 4 * qt + 4):
                    for nb in range(8):
                        n0 = nb * TT
                        k = t16 * 8 + nb

                        def op(t16=t16, n0=n0, k=k):
                            o_ps = psp.tile([128, TT], F32,
                                            tag=("fA", "fB")[k % 2], name="o_ps")
                            for fc in range(HPC):
                                lhsT = atq[(fc, t16 // 4)][
                                    :, (t16 % 4) * 128:(t16 % 4 + 1) * 128]
                                nc.tensor.matmul(
                                    o_ps[:], lhsT, wo_sb[:, fc, n0:n0 + TT],
                                    start=(fc == 0), stop=(fc == HPC - 1),
                                )
                            ob = obp.tile([128, TT], BF, tag="ob")
                            if ob_eng == "mix" and k % 2 == 0:
                                nc.vector.tensor_copy(ob[:], o_ps[:])
                            elif ob_eng == "mix":
                                nc.scalar.copy(ob[:], o_ps[:])
                            else:
                                nc.vector.tensor_copy(ob[:], o_ps[:])
                            eng = (nc.sync if ring == "sync" or k % 2 == 0
                                   else nc.scalar)
                            eng.dma_start(
                                out[t16 * 128:(t16 + 1) * 128, n0:n0 + TT], ob[:])
                        enq(op)
                mark(f"oproj{qt}")

            # ---- attention sweep ----
            sweep_no = [0]

            def sweep(h, qt):
                sid = sweep_no[0]
                sweep_no[0] += 1
                qT = qTt[(h, qt)]
                atq[(h, qt)] = atp.tile([HD, TT], BF, tag=f"at{h}",
                                        name=f"at{h}_{qt}")
                acc = accp.tile([128, TT], BF, tag=f"acc{sid % 2}",
                                name=f"acc{h}_{qt}")
                a_ps = psp.tile([HD, TT], F32, tag=f"a{sid % 2}",
                                name=f"a_ps{h}_{qt}")
                nkt = 4 * qt + 4
                # k-tiles processed in pairs sharing a [128,1024] 2-bank PSUM
                # tile so tanh/exp amortize the ~293ns/op ACT overhead; attnV
                # runs one pair behind so its et operand is always ready.
                # Diag pairs: the gap columns hold stale garbage that tanh
                # saturates to +-1 (finite) and nothing downstream reads.
                prev = None
                for p in range(nkt // 2):
                    s_w = psp.tile([128, 2 * TT], F32, tag=f"scw{p % 2}",
                                   name="s_w")
                    halves = []
                    for half in range(2):
                        kt = 2 * p + half
                        m = kt - 4 * qt
                        j0 = 128 * m if m > 0 else 0
                        off = half * TT
                        ks = kTt[kt // 4][:, (kt % 4) * 128:(kt % 4 + 1) * 128]
                        if m >= 0:
                            nc.tensor.matmul(
                                s_w[:, off + j0:off + j0 + 128], mT_sb[:],
                                mR_sb[:], start=True, stop=False)
                            nc.tensor.matmul(
                                s_w[:, off + j0:off + TT], ks, qT[:, j0:TT],
                                start=False, stop=True)
                        else:
                            nc.tensor.matmul(s_w[:, off:off + TT], ks, qT[:],
                                             start=True, stop=True)
                        halves.append((j0, kt, off))
                    ja = halves[0][0]
                    st = stp.tile([128, 2 * TT], F32, tag="st")
                    nc.scalar.activation(
                        st[:, ja:2 * TT], s_w[:, ja:2 * TT],
                        mybir.ActivationFunctionType.Tanh,
                        scale=SCALING / CAP,
                    )
                    et_w = etp.tile([128, 2 * TT], BF, tag="et")
                    nc.scalar.activation(
                        et_w[:, ja:2 * TT], st[:, ja:2 * TT],
                        mybir.ActivationFunctionType.Exp,
                        scale=CAP,
                    )
                    for (j0, kt, off) in halves:
                        if kt == 0:
                            nc.vector.tensor_copy(acc[:], et_w[:, 0:TT])
                        else:
                            nc.vector.tensor_add(
                                acc[:, j0:TT], acc[:, j0:TT],
                                et_w[:, off + j0:off + TT])
                    if prev is not None:
                        pet, ph = prev
                        for (j0, kt, off) in ph:
                            nc.tensor.matmul(
                                a_ps[:, j0:TT], vbt[kt][:],
                                pet[:, off + j0:off + TT],
                                start=(kt == 0), stop=False,
                            )
                    prev = (et_w, halves)
                    drain(3 if halves[0][1] < 4 * qt else 2)
                pet, ph = prev
                drain(1)
                for idx, (j0, kt, off) in enumerate(ph):
                    nc.tensor.matmul(
                        a_ps[:, j0:TT], vbt[kt][:], pet[:, off + j0:off + TT],
                        start=(kt == 0), stop=(idx == len(ph) - 1),
                    )
                drain(2)
                # finalize: d = ones^T acc; 1/d; broadcast; normalize into atq
                # d_ps rides the OTHER a-slot: free since the previous
                # finalize's mul, and its next writer (next sweep's pair-1
                # attnV, ~3us in) comfortably outwaits the recip read.
                d_ps = psp.tile([1, TT], F32, tag=f"a{(sid + 1) % 2}",
                                name=f"d{h}_{qt}")
                nc.tensor.matmul(d_ps[:], ones_k[:], acc[:],
                                 start=True, stop=True)
                rc = nrmp.tile([1, TT], F32, tag="rc", name=f"rc{h}_{qt}")
                nc.vector.reciprocal_approx_fast(rc[:], d_ps[:])
                bcrc = bcp.tile([128, TT], F32, tag="bcrc", name=f"bc{h}_{qt}")
                nc.gpsimd.partition_broadcast(bcrc[:], rc[:])
                nc.vector.tensor_mul(atq[(h, qt)][:], a_ps[:], bcrc[:])

            # ================= main schedule =================
            # front: qkv0 in two passes with window-0 sweeps woven between
            # (attention PE work fills the DMA-bound front)
            for h in range(HPC):
                qTt[(h, 0)] = qtp.tile([HD, TT], BF, tag=f"q{h}",
                                       name=f"qT{h}_0")
            def _dstsA():
                pfA = psp.tile([128, TT], F32, tag="fA", name="qkv0_fA")
                pfB = psp.tile([128, TT], F32, tag="fB", name="qkv0_fB")
                pw = psp.tile([128, 2 * TT], F32, tag="scw0", name="qkv0_w")
                return [pfA[:], pfB[:], pw[:, 0:TT], pw[:, TT:2 * TT]]

            def _dstsB():
                pfA = psp.tile([128, TT], F32, tag="fA", name="qkv0b_fA")
                pfB = psp.tile([128, TT], F32, tag="fB", name="qkv0b_fB")
                return [pfA[:], pfB[:]]

            qkv0_pass([4, 5, 0, 1], _dstsA)
            sweep(0, 0)
            sweep(1, 0)
            qkv0_pass([2, 3], _dstsB)
            for i in range(8):
                h_eighth_dma(1, i)
            # wo on the (otherwise idle) scalar ring; first needed ~150us in
            for j in range(2):
                nc.scalar.dma_start(
                    wo_sb[:, 2 * j:2 * j + 2, :], wo_r[:, 2 * j:2 * j + 2, :])
            qkv_items(1)
            sweep(2, 0)
            sweep(3, 0)
            oproj_items(0, "mix")

            # window 1
            qkv_items(2)
            drain_until("qkv1_g1")
            sweep(0, 1)
            sweep(1, 1)
            drain_until("qkv1_g2")
            sweep(2, 1)
            sweep(3, 1)

            # window 2. oproj(1) is deferred BEHIND qkv(3) in the FIFO so it
            # pops inside ACT-heavy window 3, which otherwise starves the PE
            # (needs atp bufs=3: atq(*,1) must survive into window 3).
            qkv_items(3)
            oproj_items(1, "dve", ring="split")
            drain_until("qkv2_g1")
            sweep(0, 2)
            sweep(1, 2)
            drain_until("qkv2_g2")
            sweep(2, 2)
            sweep(3, 2)
            oproj_items(2, "dve", ring="split")

            # window 3
            drain_until("qkv3_g1")
            sweep(0, 3)
            sweep(1, 3)
            drain_until("qkv3_g2")
            sweep(2, 3)
            sweep(3, 3)

            drain_all()
            oproj_items(3, "mix", ring="split")
            drain_all()
    return nc


_CACHE = {}


def _get_nc():
    if "nc" not in _CACHE:
        nc = bacc.Bacc("TRN2", target_bir_lowering=False, debug=False)
        _emit(nc)
        nc.compile()
        _CACHE["nc"] = nc
    return _CACHE["nc"]


def _in_maps(positions, hidden_states, w_qkv, w_o):
    bf16 = ml_dtypes.bfloat16
    hidden_states = np.asarray(hidden_states, dtype=np.float32)
    w_qkv = np.asarray(w_qkv, dtype=np.float32)
    w_o = np.asarray(w_o, dtype=np.float32)
    pos = np.asarray(positions).astype(np.float64)

    # hC[p, tt, c, t] = hidden.T[c*128+p, tt*512+t]
    hT = np.ascontiguousarray(hidden_states.T).astype(bf16)      # [D, T]
    hC = np.ascontiguousarray(
        hT.reshape(NCH, 128, NTT, TT).transpose(1, 2, 0, 3))     # [128,4,32,512]
    inv_freq = 1.0 / (10000.0 ** (np.arange(HALF, dtype=np.float64) * 2.0 / HD))
    ang = np.outer(inv_freq, pos)                      # [64, T]
    cos = np.cos(ang).astype(np.float32)
    sin = np.sin(ang).astype(np.float32)
    ccm = np.ascontiguousarray(np.concatenate([cos, cos], axis=0)).astype(bf16)
    ssm = np.ascontiguousarray(np.concatenate([-sin, sin], axis=0)).astype(bf16)
    # mask consts: out[i,t] = sum_k mT[k,i]*mR[k,t] = -2^30 iff t < i
    mTm = np.ascontiguousarray(np.triu(np.ones((128, 128)))).astype(bf16)
    mRm = np.zeros((128, 128), dtype=np.float32)
    mRm[np.arange(1, 128), np.arange(0, 127)] = -float(2 ** 30)
    mRm = np.ascontiguousarray(mRm).astype(bf16)

    in_maps = []
    for c in range(NCORES):
        rows = np.concatenate([
            w_qkv[QF * c:QF * (c + 1)],
            w_qkv[D + HD * c:D + HD * (c + 1)],
            w_qkv[D + HD * NCORES + HD * c:D + HD * NCORES + HD * (c + 1)],
        ], axis=0)                                      # [768, 4096]
        wq_c = rows.T.astype(bf16)                      # [4096, 768]
        # group-major feature order [k,v | q0,q1 | q2,q3] so each group's
        # weight columns are one contiguous DMA stream
        perm = np.r_[512:768, 0:256, 256:512]
        wqC = np.ascontiguousarray(
            wq_c[:, perm].reshape(NCH, 128, 3, 256)
            .transpose(1, 2, 0, 3))                     # [128,3,32,256]
        wo_c = w_o[:, QF * c:QF * (c + 1)].T.astype(bf16)        # [512, 4096]
        woC = np.ascontiguousarray(
            wo_c.reshape(HPC, 128, D).transpose(1, 0, 2))        # [128,4,4096]
        in_maps.append({"hC": hC, "wqG": wqC, "woC": woC,
                        "cc": ccm, "ss": ssm, "mT": mTm, "mR": mRm})
    return in_maps


def run(positions, hidden_states, w_qkv, w_o, trace=False):
    nc = _get_nc()
    in_maps = _in_maps(positions, hidden_states, w_qkv, w_o)
    res = run_bass_kernel_spmd(nc, in_maps, list(range(NCORES)), trace=trace)
    parts = np.stack([np.asarray(res.results[i]["out"], dtype=np.float32)
                      for i in range(NCORES)], axis=0)
    full = parts.sum(axis=0, dtype=np.float64).astype(np.float32)
    return full, res


def kernel(positions, hidden_states, w_qkv, w_o):
    full, _ = run(positions, hidden_states, w_qkv, w_o, trace=False)
    return full
